# revision 2
# baseline (speedup 1.0000x reference)
"""Max pairwise L2 distance between two embedding sets, on 8 Trainium2 cores.

Problem: l [8192, 64] f32, r [8192, 64] f32 -> scalar f32
    out = sqrt(max_ij ||l_i - r_j||^2)

Strategy
--------
The distance matrix has 67M entries; any exact max must examine every one.
On TRN2 the only engines that can read PSUM (where matmul output lands) are
VectorE (1 fp32/lane/cycle @ 0.96 GHz) and ScalarE (1/lane/cycle @ 1.2 GHz),
so the examination is the bottleneck, not the matmul.  We therefore:

1. On host, pick a strong candidate pair (extreme norms / extreme projections)
   and compute its exact distance L.  Pick thr = L - delta where delta bounds
   the bf16 matmul error.  Any entry <= thr cannot beat L.
2. Augment the K dimension so the PE itself computes sq_dist - thr:
      l_aug = [-2*l | lsq_hi lsq_lo 1 1 1]       (K = 69 rows, bf16)
      r_aug = [  r  | 1 1 rsq_hi rsq_lo -thr]
   (norms carried as bf16 hi+lo pairs for accuracy; thr exactly bf16).
3. Shard rows of l across the 8 cores (1024 each); every core streams all of
   r.  Each core runs 128 matmuls of [69,128]x[69,512] into 4-bank PSUM
   groups; VectorE max-reduces odd groups, ScalarE relu+sum-accumulates even
   groups.  A partition-row's partial > 0 iff some entry exceeded thr.
4. Host exactly (float64) recomputes the few flagged rows and returns
   sqrt(max(L, flagged maxima)) - an exact fp32 answer.
"""

import numpy as np
import ml_dtypes

N_CORES = 8
N_L, N_R, DIM = 8192, 8192, 64
K_AUG = 69                      # 64 dims + lsq_hi/lo + rsq_hi/lo + thr
L_COLS = N_L // N_CORES         # 1024 l-rows per core
M_TILE = 128                    # stationary free dim (l rows per matmul)
N_FREE = 512                    # moving free dim (one PSUM bank)
CHUNK = 2048                    # consumer group = 4 banks
BF16 = ml_dtypes.bfloat16

_COMPILED = {}


def _build_nc(l_cols=L_COLS, r_cols=N_R, repeats=1):
    """Build + compile the per-core SPMD program.

    Inputs : l_blk [K_AUG, l_cols] bf16, r_all [K_AUG, r_cols] bf16
    Outputs: dve_part [128, n_groups/2] f32  (max of sq-thr over group, odd g)
             act_part [128, n_groups/2] f32  (sum of relu(sq-thr), even g)
    """
    import concourse.tile as tile
    from concourse import bacc, mybir

    m_tiles = l_cols // M_TILE
    n_chunks = r_cols // CHUNK
    groups = m_tiles * n_chunks
    assert groups % 2 == 0

    nc = bacc.Bacc("TRN2", target_bir_lowering=False, debug=False,
                   num_devices=N_CORES)
    bf16 = mybir.dt.bfloat16
    f32 = mybir.dt.float32

    l_in = nc.dram_tensor("l_blk", [K_AUG, l_cols], bf16,
                          kind="ExternalInput").ap()
    r_in = nc.dram_tensor("r_all", [K_AUG, r_cols], bf16,
                          kind="ExternalInput").ap()
    dve_out = nc.dram_tensor("dve_part", [128, groups // 2], f32,
                             kind="ExternalOutput").ap()
    act_out = nc.dram_tensor("act_part", [128, groups - groups // 2], f32,
                             kind="ExternalOutput").ap()

    with tile.TileContext(nc) as tc:
        with (tc.tile_pool(name="io", bufs=1) as io_pool,
              tc.tile_pool(name="psum", bufs=2, space="PSUM") as psum_pool,
              tc.tile_pool(name="scratch", bufs=1) as scratch_pool):
            l_sb = io_pool.tile([K_AUG, l_cols], bf16)
            nc.sync.dma_start(l_sb[:], l_in[:])
            r_sb = io_pool.tile([K_AUG, r_cols], bf16)
            for ch in range(n_chunks):
                sl = slice(ch * CHUNK, (ch + 1) * CHUNK)
                nc.sync.dma_start(r_sb[:, sl], r_in[:, sl])

            dve_sb = io_pool.tile([128, groups // 2], f32)
            act_sb = io_pool.tile([128, groups - groups // 2], f32)
            scratch = scratch_pool.tile([128, CHUNK], f32)

            for _ in range(repeats):
                dve_slot = 0
                act_slot = 0
                # n-chunk outer so the first groups only need r chunk 0
                for g in range(groups):
                    ch, m = divmod(g, m_tiles)
                    ps = psum_pool.tile([128, CHUNK], f32)
                    for k in range(CHUNK // N_FREE):
                        ncol = ch * CHUNK + k * N_FREE
                        nc.tensor.matmul(
                            ps[:, k * N_FREE:(k + 1) * N_FREE],
                            l_sb[:, m * M_TILE:(m + 1) * M_TILE],
                            r_sb[:, ncol:ncol + N_FREE],
                            start=True, stop=True)
                    if g % 2 == 1:
                        nc.vector.tensor_reduce(
                            dve_sb[:, dve_slot:dve_slot + 1], ps[:, :],
                            axis=mybir.AxisListType.X, op=mybir.AluOpType.max)
                        dve_slot += 1
                    else:
                        nc.scalar.activation(
                            scratch[:, :], ps[:, :],
                            mybir.ActivationFunctionType.Relu,
                            accum_out=act_sb[:, act_slot:act_slot + 1])
                        act_slot += 1

            nc.sync.dma_start(dve_out[:], dve_sb[:])
            nc.sync.dma_start(act_out[:], act_sb[:])

    nc.compile()
    return nc


def _get_nc(key=("full", 1)):
    if key not in _COMPILED:
        kind, repeats = key
        _COMPILED[key] = _build_nc(repeats=repeats)
    return _COMPILED[key]


def _candidate_threshold(l64, r64, ln, rn):
    """Exact (float64) max squared distance over a cheap candidate set."""
    cl = set(np.argsort(-ln)[:64].tolist())
    cr = set(np.argsort(-rn)[:64].tolist())
    rng = np.random.default_rng(12345)
    U = rng.standard_normal((16, DIM))
    U /= np.linalg.norm(U, axis=1, keepdims=True)
    pl = l64 @ U.T
    pr = r64 @ U.T
    for k in range(U.shape[0]):
        cl.update(np.argsort(-pl[:, k])[:8].tolist())
        cl.update(np.argsort(pl[:, k])[:8].tolist())
        cr.update(np.argsort(-pr[:, k])[:8].tolist())
        cr.update(np.argsort(pr[:, k])[:8].tolist())
    A = l64[sorted(cl)]
    B = r64[sorted(cr)]
    d2 = ((A * A).sum(1)[:, None] + (B * B).sum(1)[None, :]
          - 2.0 * (A @ B.T))
    return float(d2.max())


def _hi_lo_bf16(x64):
    hi = x64.astype(np.float32).astype(BF16)
    lo = (x64 - hi.astype(np.float64)).astype(np.float32).astype(BF16)
    return hi, lo


def _prepare_inputs(l, r):
    """Returns (l_aug [K_AUG, N_L] bf16, r_aug [K_AUG, N_R] bf16, L, thr)."""
    l64 = l.astype(np.float64)
    r64 = r.astype(np.float64)
    lsq = (l64 * l64).sum(1)
    rsq = (r64 * r64).sum(1)
    ln = np.sqrt(lsq)
    rn = np.sqrt(rsq)

    L = _candidate_threshold(l64, r64, ln, rn)
    # bf16 error bound on device sq-dist: cross term 2^-8 * 2*|l||r|, plus
    # slack for norm hi/lo rounding and fp32 accumulation.
    delta = 2.0 ** -8 * 2.0 * float(ln.max()) * float(rn.max()) + 0.05
    thr = float(np.asarray(L - delta, dtype=np.float32).astype(BF16))

    lsq_hi, lsq_lo = _hi_lo_bf16(lsq)
    rsq_hi, rsq_lo = _hi_lo_bf16(rsq)

    l_aug = np.zeros((K_AUG, N_L), dtype=BF16)
    l_aug[:DIM] = (-2.0 * l.astype(np.float32).T).astype(BF16)
    l_aug[64] = lsq_hi
    l_aug[65] = lsq_lo
    l_aug[66] = BF16(1.0)
    l_aug[67] = BF16(1.0)
    l_aug[68] = BF16(1.0)

    r_aug = np.zeros((K_AUG, N_R), dtype=BF16)
    r_aug[:DIM] = r.astype(np.float32).T.astype(BF16)
    r_aug[64] = BF16(1.0)
    r_aug[65] = BF16(1.0)
    r_aug[66] = rsq_hi
    r_aug[67] = rsq_lo
    r_aug[68] = BF16(-thr)

    return np.ascontiguousarray(l_aug), np.ascontiguousarray(r_aug), L, thr


def _run_device(l_aug, r_aug, nc=None):
    from concourse.bass_utils import run_bass_kernel_spmd
    if nc is None:
        nc = _get_nc()
    in_maps = [
        {"l_blk": np.ascontiguousarray(l_aug[:, c * L_COLS:(c + 1) * L_COLS]),
         "r_all": r_aug}
        for c in range(N_CORES)
    ]
    res = run_bass_kernel_spmd(nc, in_maps, core_ids=list(range(N_CORES)))
    return res.results


def kernel(l_dfa_embeddings, r_dfa_embeddings):
    l = np.asarray(l_dfa_embeddings, dtype=np.float32)
    r = np.asarray(r_dfa_embeddings, dtype=np.float32)
    assert l.shape == (N_L, DIM) and r.shape == (N_R, DIM)

    l_aug, r_aug, L, thr = _prepare_inputs(l, r)
    results = _run_device(l_aug, r_aug)

    l64 = l.astype(np.float64)
    r64 = r.astype(np.float64)
    rsq = (r64 * r64).sum(1)

    m_tiles = L_COLS // M_TILE
    best = L
    for c in range(N_CORES):
        dve = results[c]["dve_part"]
        act = results[c]["act_part"]
        for g in range(m_tiles * (N_R // CHUNK)):
            part = dve[:, g // 2] if g % 2 == 1 else act[:, g // 2]
            flagged = np.nonzero(part > 0.0)[0]
            if flagged.size == 0:
                continue
            ch, m = divmod(g, m_tiles)
            cols = slice(ch * CHUNK, (ch + 1) * CHUNK)
            for p in flagged:
                lrow = c * L_COLS + m * M_TILE + int(p)
                d2 = ((l64[lrow] * l64[lrow]).sum() + rsq[cols]
                      - 2.0 * (r64[cols] @ l64[lrow]))
                best = max(best, float(d2.max()))

    return np.float32(np.sqrt(max(best, 0.0)))


# revision 10
# speedup vs baseline: 5.1096x; 5.1096x over previous
"""Max pairwise L2 distance between two embedding sets, on 8 Trainium2 cores.

Problem: l [8192, 64] f32, r [8192, 64] f32 -> scalar f32
    out = sqrt(max_ij ||l_i - r_j||^2)

Strategy
--------
The distance matrix has 67M entries; any exact max must examine every one.
On TRN2 the only engines that can read PSUM (where matmul output lands) are
VectorE (1 fp32/lane/cycle @ 0.96 GHz) and ScalarE (1/lane/cycle @ 1.2 GHz),
so the examination is the bottleneck, not the matmul.  We therefore:

1. On host, pick a strong candidate pair (extreme norms / extreme projections)
   and compute its exact distance L.  Pick thr = L - delta where delta bounds
   the bf16 matmul error.  Any entry <= thr cannot beat L.
2. Augment the K dimension so the PE itself computes sq_dist - thr:
      l_aug = [-2*l | lsq_hi lsq_lo 1 1 1]       (K = 69 rows, bf16)
      r_aug = [  r  | 1 1 rsq_hi rsq_lo -thr]
   (norms carried as bf16 hi+lo pairs for accuracy; thr exactly bf16).
3. Shard rows of l across the 8 cores (1024 each); every core streams all of
   r.  Each core runs 128 matmuls of [69,128]x[69,512] into 4-bank PSUM
   groups; VectorE max-reduces odd groups, ScalarE relu+sum-accumulates even
   groups.  A partition-row's partial > 0 iff some entry exceeded thr.
4. Host exactly (float64) recomputes the few flagged rows and returns
   sqrt(max(L, flagged maxima)) - an exact fp32 answer.
"""

import numpy as np
import ml_dtypes

N_CORES = 8
N_L, N_R, DIM = 8192, 8192, 64
K_AUG = 69                      # 64 dims + lsq_hi/lo + rsq_hi/lo + thr
L_COLS = N_L // N_CORES         # 1024 l-rows per core
M_TILE = 128                    # stationary free dim (l rows per matmul)
N_FREE = 512                    # moving free dim (one PSUM bank)
CHUNK = 1024                    # consumer group = 2 banks
BF16 = ml_dtypes.bfloat16

_COMPILED = {}


def _assignment(groups):
    """Bresenham ACT/DVE split (cost-model claim costs are ~equal: ACT
    ~1183ns incl. accumulator read vs DVE ~1192ns). Returns list of bools
    (True = ACT)."""
    n_act = round(groups * 32 / 64)
    out = []
    acc = 0
    for _ in range(groups):
        acc += n_act
        if acc >= groups:
            acc -= groups
            out.append(True)
        else:
            out.append(False)
    assert sum(out) == n_act
    return out


def _build_nc(l_cols=L_COLS, r_cols=N_R, repeats=1):
    """Build + compile the per-core SPMD program.

    Inputs : l_blk [K_AUG, l_cols] bf16, r_all [K_AUG, r_cols] bf16
    Outputs: dve_part [128, n_groups/2] f32  (max of sq-thr over group, odd g)
             act_part [128, n_groups/2] f32  (sum of relu(sq-thr), even g)
    """
    import concourse.tile as tile
    from concourse import bacc, mybir

    m_tiles = l_cols // M_TILE
    n_chunks = r_cols // CHUNK
    groups = m_tiles * n_chunks
    assign_act = _assignment(groups)
    n_act = sum(assign_act)
    n_dve = groups - n_act

    nc = bacc.Bacc("TRN2", target_bir_lowering=False, debug=False,
                   num_devices=N_CORES)
    bf16 = mybir.dt.bfloat16
    f32 = mybir.dt.float32

    l_in = nc.dram_tensor("l_blk", [K_AUG, l_cols], bf16,
                          kind="ExternalInput").ap()
    r_in = nc.dram_tensor("r_all", [K_AUG, r_cols], bf16,
                          kind="ExternalInput").ap()
    dve_out = nc.dram_tensor("dve_part", [128, n_dve], f32,
                             kind="ExternalOutput").ap()
    act_out = nc.dram_tensor("act_part", [128, n_act], f32,
                             kind="ExternalOutput").ap()

    with tile.TileContext(nc) as tc:
        with (tc.tile_pool(name="io", bufs=1) as io_pool,
              tc.tile_pool(name="psum", bufs=4, space="PSUM") as psum_pool,
              tc.tile_pool(name="scratch", bufs=1) as scratch_pool):
            # tiny dummy activation first so the ACT table set loads during
            # the DMA prologue instead of before the first real group
            warm = scratch_pool.tile([128, 1], f32)
            nc.vector.memset(warm[:], 0.0)
            nc.scalar.activation(warm[:], warm[:],
                                 mybir.ActivationFunctionType.Relu)

            l_sb = io_pool.tile([K_AUG, l_cols], bf16)
            # first m-tiles land first so group 0 starts ASAP
            nc.sync.dma_start(l_sb[:, :2 * M_TILE], l_in[:, :2 * M_TILE])
            nc.sync.dma_start(l_sb[:, 2 * M_TILE:], l_in[:, 2 * M_TILE:])
            r_sb = io_pool.tile([K_AUG, r_cols], bf16)
            for ch in range(n_chunks):
                sl = slice(ch * CHUNK, (ch + 1) * CHUNK)
                nc.sync.dma_start(r_sb[:, sl], r_in[:, sl])

            dve_sb = io_pool.tile([128, n_dve], f32)
            act_sb = io_pool.tile([128, n_act], f32)

            for _ in range(repeats):
                dve_slot = 0
                act_slot = 0
                # n-chunk outer so the first groups only need r chunk 0
                for g in range(groups):
                    ch, m = divmod(g, m_tiles)
                    ps = psum_pool.tile([128, CHUNK], f32)
                    for k in range(CHUNK // N_FREE):
                        ncol = ch * CHUNK + k * N_FREE
                        nc.tensor.matmul(
                            ps[:, k * N_FREE:(k + 1) * N_FREE],
                            l_sb[:, m * M_TILE:(m + 1) * M_TILE],
                            r_sb[:, ncol:ncol + N_FREE],
                            start=True, stop=True)
                    if assign_act[g]:
                        # relu in place in PSUM (ScalarE is closest to PSUM;
                        # next matmul start=True clears has_written anyway)
                        nc.scalar.activation(
                            ps[:, :], ps[:, :],
                            mybir.ActivationFunctionType.Relu,
                            accum_out=act_sb[:, act_slot:act_slot + 1])
                        act_slot += 1
                    else:
                        nc.vector.tensor_reduce(
                            dve_sb[:, dve_slot:dve_slot + 1], ps[:, :],
                            axis=mybir.AxisListType.X, op=mybir.AluOpType.max)
                        dve_slot += 1

            nc.sync.dma_start(dve_out[:], dve_sb[:])
            nc.sync.dma_start(act_out[:], act_sb[:])

    nc.compile()
    return nc


def _get_nc(key=("full", 1)):
    if key not in _COMPILED:
        kind, repeats = key
        _COMPILED[key] = _build_nc(repeats=repeats)
    return _COMPILED[key]


def _candidate_threshold(l64, r64, ln, rn):
    """Exact (float64) max squared distance over a cheap candidate set."""
    cl = set(np.argsort(-ln)[:64].tolist())
    cr = set(np.argsort(-rn)[:64].tolist())
    rng = np.random.default_rng(12345)
    U = rng.standard_normal((16, DIM))
    U /= np.linalg.norm(U, axis=1, keepdims=True)
    pl = l64 @ U.T
    pr = r64 @ U.T
    for k in range(U.shape[0]):
        cl.update(np.argsort(-pl[:, k])[:8].tolist())
        cl.update(np.argsort(pl[:, k])[:8].tolist())
        cr.update(np.argsort(-pr[:, k])[:8].tolist())
        cr.update(np.argsort(pr[:, k])[:8].tolist())
    A = l64[sorted(cl)]
    B = r64[sorted(cr)]
    d2 = ((A * A).sum(1)[:, None] + (B * B).sum(1)[None, :]
          - 2.0 * (A @ B.T))
    return float(d2.max())


def _hi_lo_bf16(x64):
    hi = x64.astype(np.float32).astype(BF16)
    lo = (x64 - hi.astype(np.float64)).astype(np.float32).astype(BF16)
    return hi, lo


def _prepare_inputs(l, r):
    """Returns (l_aug [K_AUG, N_L] bf16, r_aug [K_AUG, N_R] bf16, L, thr)."""
    l64 = l.astype(np.float64)
    r64 = r.astype(np.float64)
    lsq = (l64 * l64).sum(1)
    rsq = (r64 * r64).sum(1)
    ln = np.sqrt(lsq)
    rn = np.sqrt(rsq)

    L = _candidate_threshold(l64, r64, ln, rn)
    # bf16 error bound on device sq-dist: cross term 2^-8 * 2*|l||r|, plus
    # slack for norm hi/lo rounding and fp32 accumulation.
    delta = 2.0 ** -8 * 2.0 * float(ln.max()) * float(rn.max()) + 0.05
    thr = float(np.asarray(L - delta, dtype=np.float32).astype(BF16))

    lsq_hi, lsq_lo = _hi_lo_bf16(lsq)
    rsq_hi, rsq_lo = _hi_lo_bf16(rsq)

    l_aug = np.zeros((K_AUG, N_L), dtype=BF16)
    l_aug[:DIM] = (-2.0 * l.astype(np.float32).T).astype(BF16)
    l_aug[64] = lsq_hi
    l_aug[65] = lsq_lo
    l_aug[66] = BF16(1.0)
    l_aug[67] = BF16(1.0)
    l_aug[68] = BF16(1.0)

    r_aug = np.zeros((K_AUG, N_R), dtype=BF16)
    r_aug[:DIM] = r.astype(np.float32).T.astype(BF16)
    r_aug[64] = BF16(1.0)
    r_aug[65] = BF16(1.0)
    r_aug[66] = rsq_hi
    r_aug[67] = rsq_lo
    r_aug[68] = BF16(-thr)

    return np.ascontiguousarray(l_aug), np.ascontiguousarray(r_aug), L, thr


def _run_device(l_aug, r_aug, nc=None):
    from concourse.bass_utils import run_bass_kernel_spmd
    if nc is None:
        nc = _get_nc()
    in_maps = [
        {"l_blk": np.ascontiguousarray(l_aug[:, c * L_COLS:(c + 1) * L_COLS]),
         "r_all": r_aug}
        for c in range(N_CORES)
    ]
    res = run_bass_kernel_spmd(nc, in_maps, core_ids=list(range(N_CORES)))
    return res.results


def kernel(l_dfa_embeddings, r_dfa_embeddings):
    l = np.asarray(l_dfa_embeddings, dtype=np.float32)
    r = np.asarray(r_dfa_embeddings, dtype=np.float32)
    assert l.shape == (N_L, DIM) and r.shape == (N_R, DIM)

    l_aug, r_aug, L, thr = _prepare_inputs(l, r)
    results = _run_device(l_aug, r_aug)

    l64 = l.astype(np.float64)
    r64 = r.astype(np.float64)
    rsq = (r64 * r64).sum(1)

    m_tiles = L_COLS // M_TILE
    groups = m_tiles * (N_R // CHUNK)
    assign_act = _assignment(groups)
    best = L
    for c in range(N_CORES):
        dve = results[c]["dve_part"]
        act = results[c]["act_part"]
        dve_slot = act_slot = 0
        for g in range(groups):
            if assign_act[g]:
                part = act[:, act_slot]
                act_slot += 1
            else:
                part = dve[:, dve_slot]
                dve_slot += 1
            flagged = np.nonzero(part > 0.0)[0]
            if flagged.size == 0:
                continue
            ch, m = divmod(g, m_tiles)
            cols = slice(ch * CHUNK, (ch + 1) * CHUNK)
            for p in flagged:
                lrow = c * L_COLS + m * M_TILE + int(p)
                d2 = ((l64[lrow] * l64[lrow]).sum() + rsq[cols]
                      - 2.0 * (r64[cols] @ l64[lrow]))
                best = max(best, float(d2.max()))

    return np.float32(np.sqrt(max(best, 0.0)))
